# revision 8
# baseline (speedup 1.0000x reference)
"""Trainium2 Bass kernel for 2D Gaussian Splatting (N=1024 gaussians, 256x256).

Math: sigma[p,i] is a quadratic polynomial in pixel coords, so with a
block-centered pixel basis F[12,128] (6 monomials, each duplicated for a
hi/lo bf16 coefficient split) m1 = log(op) - sigma is ONE bf16 matmul
F.T @ G per 512-column chunk (the basis is block-independent in local
coords, so a single lhsT serves the whole stream). alpha = exp(m1) on
the scalar engine; a = alpha - 1 on gpsimd/DVE; compositing uses the
shifted state D = C - c (c = per-gaussian color) which obeys
    D_k = (delta_k - D_{k-1}) * a_k,   delta_k = c_k - c_{k-1},
one DVE tensor_tensor_scan (op0=subtract, op1=mult) per chunk with no
separate beta/bt tensors. delta is a host-precomputed constant. The host
adds back c at each slot's final column.

Culling: image split into 512 blocks of 8x16 pixels; a gaussian is kept
for a block iff its exact minimal sigma over the block is < 8 (dropped
alpha contributions < 2e-4 rel). Blocks are snake-dealt by surviving
count onto the 8 cores (SPMD: identical slot schedule, data-dependent
content only). Slots are front-padded with sentinel columns (all-zero
coefficients -> m1=0 -> a=0 resets the scan) to a multiple of 16
columns and bin-packed into independent 512-column chunks; only slots
wider than 512 chain scan state across their own consecutive chunks.
Slot composites are extracted with one strided copy (every 16th column)
+ one DMA; the host picks each slot's end column.

Sharding: 8 NeuronCores; gaussian params replicated, blocks balanced;
host reassembles the image from the per-core strided outputs.
"""

import numpy as np
import ml_dtypes

H = 256
W = 256
N = 1024
NCORES = 8
BR, BC = 8, 16                 # block = 8 rows x 16 cols = 128 pixels
NBY, NBX = H // BR, W // BC
NBLK = NBY * NBX               # 512
SLOTS = NBLK // NCORES         # 64 slots per core
CULL_T = 6.0
QU = 8                         # unit quantum (columns); slot ends at u*QU-1
CAP = 512 // QU                # units per chunk
EPS2D = 0.3

_cache = {}


# ---------------------------------------------------------------- host math

def _preprocess(means, quats, scales, rgbs, opacities, viewmat, K):
    """Float64 per-gaussian preprocessing. Returns, in back-to-front order:
    conic (ca, cb, cc), pixel means (u, v), log-opacity lop, colors colv."""
    md = means.astype(np.float64)
    Rv = viewmat[:3, :3].astype(np.float64)
    t = viewmat[:3, 3].astype(np.float64)
    p_cam = md @ Rv.T + t
    x, y, z = p_cam[:, 0], p_cam[:, 1], p_cam[:, 2]
    fx, fy = float(K[0, 0]), float(K[1, 1])
    cx, cy = float(K[0, 2]), float(K[1, 2])
    inv_z = 1.0 / z
    u = fx * x * inv_z + cx
    v = fy * y * inv_z + cy

    th = quats.astype(np.float64)
    ct, st = np.cos(th), np.sin(th)
    zr = np.zeros_like(ct)
    R3 = np.stack([np.stack([ct, -st, zr], -1),
                   np.stack([st, ct, zr], -1),
                   np.stack([zr, zr, np.ones_like(ct)], -1)], -2)
    M = R3 * scales.astype(np.float64)[:, None, :]
    cov3 = M @ np.swapaxes(M, -1, -2)
    cov_cam = np.einsum('ij,njk,lk->nil', Rv, cov3, Rv)
    j0 = np.stack([fx * inv_z, zr, -fx * x * inv_z * inv_z], -1)
    j1 = np.stack([zr, fy * inv_z, -fy * y * inv_z * inv_z], -1)
    J = np.stack([j0, j1], -2)
    cov2 = np.einsum('nij,njk,nlk->nil', J, cov_cam, J)
    a = cov2[:, 0, 0] + EPS2D
    b = cov2[:, 0, 1]
    c = cov2[:, 1, 1] + EPS2D
    det = a * c - b * b
    ca, cb, cc = c / det, -b / det, a / det

    op = 1.0 / (1.0 + np.exp(-opacities.astype(np.float64)))
    colv = 1.0 / (1.0 + np.exp(-rgbs.astype(np.float64)[:, 0]))

    # reference sorts by fp32 camera z ascending (stable); we composite
    # back-to-front = exact reverse
    order = np.argsort(z.astype(np.float32), kind="stable")
    rev = order[::-1]
    return (ca[rev], cb[rev], cc[rev], u[rev], v[rev],
            np.log(op)[rev], colv[rev])


def _block_masks(ca, cb, cc, u, v):
    """Exact minimal sigma over each block rectangle: 0 if the center is
    inside, else the min over the four edges (1D quadratic, clamped)."""
    def sigma_at(dx, dy):
        return 0.5 * ca * dx * dx + cb * dx * dy + 0.5 * cc * dy * dy

    masks = np.zeros((NBLK, N), bool)
    for by in range(NBY):
        y0, y1 = by * BR + 0.5, by * BR + BR - 0.5
        for bx in range(NBX):
            x0, x1 = bx * BC + 0.5, bx * BC + BC - 0.5
            smin = np.full(N, np.inf)
            for xe in (x0, x1):
                dxe = xe - u
                dye = np.clip(-cb * dxe / cc, y0 - v, y1 - v)
                smin = np.minimum(smin, sigma_at(dxe, dye))
            for ye in (y0, y1):
                dye = ye - v
                dxe = np.clip(-cb * dye / ca, x0 - u, x1 - u)
                smin = np.minimum(smin, sigma_at(dxe, dye))
            inside = (u >= x0) & (u <= x1) & (v >= y0) & (v <= y1)
            smin[inside] = 0.0
            masks[by * NBX + bx] = smin < CULL_T
    return masks


def _local_basis():
    """[12, 128] block-local monomial basis (exact in bf16)."""
    px = np.arange(BC) + 0.5 - BC / 2.0          # -7.5 .. 7.5
    py = np.arange(BR) + 0.5 - BR / 2.0          # -3.5 .. 3.5
    gx, gy = np.meshgrid(px, py)                 # [BR, BC] row-major
    fx, fy = gx.ravel(), gy.ravel()
    rows = [fx * fx, fx * fy, fy * fy, fx, fy, np.ones_like(fx)]
    return np.repeat(np.stack(rows, 0), 2, axis=0)  # each row duplicated


def _build_schedule(ca, cb, cc, u, v, lop, colv):
    """Cull per block, snake-deal blocks to cores, bin-pack padded slots
    into independent 512-col chunks, build per-core device arrays."""
    masks = _block_masks(ca, cb, cc, u, v)
    widths = masks.sum(1)

    order = np.argsort(widths, kind="stable")[::-1]
    blk_of = np.zeros((NCORES, SLOTS), np.int32)
    for j in range(SLOTS):
        grp = order[j * NCORES:(j + 1) * NCORES]
        if j % 2 == 1:
            grp = grp[::-1]
        blk_of[:, j] = grp
    slot_w = widths[blk_of].max(0)                       # shared schedule
    units = (slot_w + 1 + QU - 1) // QU                  # >=1 sentinel col

    # --- pack: oversize slots get dedicated consecutive chunks (scan
    # carries across them); regular slots first-fit-decreasing into bins
    desc = sorted(range(SLOTS), key=lambda j: -units[j])
    place = np.zeros(SLOTS, np.int64)                    # absolute unit start
    carry = []                                           # per chunk
    abs_u = 0
    regular = []
    for j in desc:
        uj = int(units[j])
        if uj > CAP:
            run = -(-uj // CAP)
            place[j] = abs_u + run * CAP - uj
            carry.extend([False] + [True] * (run - 1))
            abs_u += run * CAP
        else:
            regular.append(j)
    fills, members = [], []                              # per bin
    for j in regular:
        uj = int(units[j])
        for i in range(len(fills)):
            if fills[i] + uj <= CAP:
                place[j] = fills[i]                      # offset; base later
                members[i].append(j)
                fills[i] += uj
                break
        else:
            place[j] = 0
            members.append([j])
            fills.append(uj)
    for i in range(len(fills)):
        for j in members[i]:
            place[j] += abs_u + i * CAP
        carry.append(False)
    abs_u += len(fills) * CAP
    Lpad = abs_u * QU
    S = Lpad // 512
    assert len(carry) == S

    bf16 = ml_dtypes.bfloat16
    cores = []
    c_last = np.zeros((NCORES, SLOTS), np.float64)
    for cid in range(NCORES):
        g12 = np.zeros((12, Lpad), np.float64)
        delta = np.zeros(Lpad, np.float32)
        for j in range(SLOTS):
            blk = int(blk_of[cid, j])
            idx = np.nonzero(masks[blk])[0]
            nb = len(idx)
            if nb == 0:
                continue
            end = (int(place[j]) + int(units[j])) * QU
            s0 = end - nb
            by, bx = divmod(blk, NBX)
            cxb = bx * BC + BC / 2.0
            cyb = by * BR + BR / 2.0
            du = u[idx] - cxb
            dv = v[idx] - cyb
            cai, cbi, cci = ca[idx], cb[idx], cc[idx]
            gs = [-0.5 * cai, -cbi, -0.5 * cci,
                  cai * du + cbi * dv, cbi * du + cci * dv,
                  lop[idx] - (0.5 * cai * du * du + cbi * du * dv
                              + 0.5 * cci * dv * dv)]
            for r, g in enumerate(gs):
                hi = np.asarray(g, dtype=bf16).astype(np.float64)
                lo = g - hi
                g12[2 * r, s0:end] = hi
                g12[2 * r + 1, s0:end] = lo
            cv = colv[idx]
            delta[s0:end] = (cv - np.concatenate([[0.0], cv[:-1]])
                             ).astype(np.float32)
            c_last[cid, j] = cv[-1]
        cores.append({"g12": g12.astype(bf16), "delta": delta})
    sched = {"blk_of": blk_of, "place": place, "units": units,
             "carry": tuple(carry), "Lpad": int(Lpad), "c_last": c_last}
    return sched, cores


# ---------------------------------------------------------------- device

def _build_module(Lpad, carry, reps=1, loop_n=1, stagger=True):
    import contextlib
    import concourse.bass as bass
    import concourse.bacc as bacc
    import concourse.tile as tile
    from concourse import mybir

    f32 = mybir.dt.float32
    bf16 = mybir.dt.bfloat16
    S = Lpad // 512
    U = Lpad // QU

    nc = bacc.Bacc(None)
    ft = nc.dram_tensor("ft", [12, 128], bf16, kind="ExternalInput")
    g12 = nc.dram_tensor("g12", [12, Lpad], bf16, kind="ExternalInput")
    delta = nc.dram_tensor("delta", [Lpad], f32, kind="ExternalInput")
    out = nc.dram_tensor("out", [128 * U], f32, kind="ExternalOutput")

    with tile.TileContext(nc) as tc:
        with (
            tc.tile_pool(name="const", bufs=1) as consts,
            tc.tile_pool(name="work", bufs=6) as work,
            tc.tile_pool(name="psum", bufs=8, space="PSUM") as psum,
        ):
            ft_s = consts.tile([12, 128], bf16)
            nc.sync.dma_start(out=ft_s[:], in_=ft[:, :])
            g_s = consts.tile([12, Lpad], bf16)
            nc.sync.dma_start(out=g_s[:], in_=g12[:, :])
            d_s = consts.tile([128, Lpad], f32)
            step = Lpad // 8
            for q in range(8):
                seg = delta[q * step:(q + 1) * step]
                bc = bass.AP(tensor=seg.tensor, offset=seg.offset,
                             ap=[[0, 128], seg.ap[0]])
                nc.sync.dma_start(out=d_s[:, q * step:(q + 1) * step], in_=bc)
            D = consts.tile([128, Lpad], f32)
            res = consts.tile([128, U], f32)

            loop_cm = (
                tc.For_i(0, loop_n, 1, hint_engines=(
                    mybir.EngineType.PE, mybir.EngineType.Activation,
                    mybir.EngineType.DVE, mybir.EngineType.Pool),
                    staggered_reset=stagger)
                if loop_n > 1 else contextlib.nullcontext()
            )
            with loop_cm:
                for _ in range(reps):
                    for s in range(S):
                        sl = slice(s * 512, (s + 1) * 512)
                        m_ps = psum.tile([128, 512], f32)
                        nc.tensor.matmul(
                            m_ps[:, :], lhsT=ft_s[:, :], rhs=g_s[:, sl],
                            start=True, stop=True,
                        )
                        a_t = work.tile([128, 512], f32)
                        nc.scalar.activation(
                            out=a_t[:], in_=m_ps[:, :],
                            func=mybir.ActivationFunctionType.Exp,
                            scale=1.0, bias=0.0,
                        )
                        am_t = work.tile([128, 512], f32)
                        eng = nc.gpsimd if s % 3 != 2 else nc.vector
                        eng.tensor_scalar_add(am_t[:], a_t[:], -1.0)
                        init = (D[:, s * 512 - 1:s * 512] if carry[s]
                                else 0.0)
                        nc.vector.tensor_tensor_scan(
                            D[:, sl], d_s[:, sl], am_t[:], init,
                            op0=mybir.AluOpType.subtract,
                            op1=mybir.AluOpType.mult,
                        )
                    dfull = D[:]
                    strided = bass.AP(
                        tensor=dfull.tensor, offset=dfull.offset + (QU - 1),
                        ap=[dfull.ap[0], [QU, U]])
                    nc.scalar.copy(out=res[:], in_=strided)
                    nc.sync.dma_start(
                        out=out[:].rearrange("(k c) -> k c", c=U),
                        in_=res[:])
    nc.finalize()
    return nc


# ---------------------------------------------------------------- entry

def _prepare(inputs, reps=1, loop_n=1, variant=None, stagger=True):
    ca, cb, cc, u, v, lop, colv = _preprocess(**inputs)
    sched, cores = _build_schedule(ca, cb, cc, u, v, lop, colv)
    key = (sched["carry"], sched["Lpad"], reps, loop_n, stagger)
    if key not in _cache:
        _cache[key] = _build_module(
            sched["Lpad"], sched["carry"], reps=reps, loop_n=loop_n,
            stagger=stagger)
    nc = _cache[key]
    ftb = _local_basis().astype(ml_dtypes.bfloat16)
    in_maps = [{"ft": ftb, "g12": cores[cid]["g12"],
                "delta": cores[cid]["delta"]} for cid in range(NCORES)]
    return nc, in_maps, sched


def _assemble(results, sched):
    img = np.zeros((H, W), np.float32)
    blk_of = sched["blk_of"]
    place, units = sched["place"], sched["units"]
    U = sched["Lpad"] // QU
    for cid in range(NCORES):
        res = results[cid]["out"].reshape(128, U)
        for j in range(SLOTS):
            by, bx = divmod(int(blk_of[cid, j]), NBX)
            uend = int(place[j]) + int(units[j]) - 1
            col = res[:, uend] + np.float32(sched["c_last"][cid, j])
            img[by * BR:(by + 1) * BR, bx * BC:(bx + 1) * BC] = (
                col.reshape(BR, BC))
    return img.reshape(1, 1, H, W)


def kernel(**inputs):
    from concourse.bass_utils import run_bass_kernel_spmd

    inputs = {k: np.asarray(v) for k, v in inputs.items()}
    nc, in_maps, sched = _prepare(inputs)
    res = run_bass_kernel_spmd(nc, in_maps, core_ids=list(range(NCORES)))
    return _assemble(res.results, sched)


# revision 10
# speedup vs baseline: 3.1587x; 3.1587x over previous
"""Trainium2 Bass kernel for 2D Gaussian Splatting (N=1024 gaussians, 256x256).

Math: sigma[p,i] is a quadratic polynomial in pixel coords, so with a
block-centered pixel basis F[12,128] (6 monomials, each duplicated for a
hi/lo bf16 coefficient split) m1 = log(op) - sigma is ONE bf16 matmul
F.T @ G per 512-column chunk (the basis is block-independent in local
coords, so a single lhsT serves the whole stream). alpha = exp(m1) on
the scalar engine; a = alpha - 1 on gpsimd/DVE; compositing uses the
shifted state D = C - c (c = per-gaussian color) which obeys
    D_k = (delta_k - D_{k-1}) * a_k,   delta_k = c_k - c_{k-1},
one DVE tensor_tensor_scan (op0=subtract, op1=mult) per chunk with no
separate beta/bt tensors. delta is a host-precomputed constant. The host
adds back c at each slot's final column.

Culling: image split into 512 blocks of 8x16 pixels; a gaussian is kept
for a block iff its exact minimal sigma over the block is < 8 (dropped
alpha contributions < 2e-4 rel). Blocks are snake-dealt by surviving
count onto the 8 cores (SPMD: identical slot schedule, data-dependent
content only). Slots are front-padded with sentinel columns (all-zero
coefficients -> m1=0 -> a=0 resets the scan) to a multiple of 16
columns and bin-packed into independent 512-column chunks; only slots
wider than 512 chain scan state across their own consecutive chunks.
Slot composites are extracted with one strided copy (every 16th column)
+ one DMA; the host picks each slot's end column.

Sharding: 8 NeuronCores; gaussian params replicated, blocks balanced;
host reassembles the image from the per-core strided outputs.
"""

import numpy as np
import ml_dtypes

H = 256
W = 256
N = 1024
NCORES = 8
BR, BC = 8, 16                 # block = 8 rows x 16 cols = 128 pixels
NBY, NBX = H // BR, W // BC
NBLK = NBY * NBX               # 512
SLOTS = NBLK // NCORES         # 64 slots per core
CULL_T = 5.0
QU = 8                         # unit quantum (columns); slot ends at u*QU-1
CAP = 512 // QU                # units per chunk
EPS2D = 0.3

_cache = {}


# ---------------------------------------------------------------- host math

def _preprocess(means, quats, scales, rgbs, opacities, viewmat, K):
    """Float64 per-gaussian preprocessing. Returns, in back-to-front order:
    conic (ca, cb, cc), pixel means (u, v), log-opacity lop, colors colv."""
    md = means.astype(np.float64)
    Rv = viewmat[:3, :3].astype(np.float64)
    t = viewmat[:3, 3].astype(np.float64)
    p_cam = md @ Rv.T + t
    x, y, z = p_cam[:, 0], p_cam[:, 1], p_cam[:, 2]
    fx, fy = float(K[0, 0]), float(K[1, 1])
    cx, cy = float(K[0, 2]), float(K[1, 2])
    inv_z = 1.0 / z
    u = fx * x * inv_z + cx
    v = fy * y * inv_z + cy

    th = quats.astype(np.float64)
    ct, st = np.cos(th), np.sin(th)
    zr = np.zeros_like(ct)
    R3 = np.stack([np.stack([ct, -st, zr], -1),
                   np.stack([st, ct, zr], -1),
                   np.stack([zr, zr, np.ones_like(ct)], -1)], -2)
    M = R3 * scales.astype(np.float64)[:, None, :]
    cov3 = M @ np.swapaxes(M, -1, -2)
    cov_cam = np.einsum('ij,njk,lk->nil', Rv, cov3, Rv)
    j0 = np.stack([fx * inv_z, zr, -fx * x * inv_z * inv_z], -1)
    j1 = np.stack([zr, fy * inv_z, -fy * y * inv_z * inv_z], -1)
    J = np.stack([j0, j1], -2)
    cov2 = np.einsum('nij,njk,nlk->nil', J, cov_cam, J)
    a = cov2[:, 0, 0] + EPS2D
    b = cov2[:, 0, 1]
    c = cov2[:, 1, 1] + EPS2D
    det = a * c - b * b
    ca, cb, cc = c / det, -b / det, a / det

    op = 1.0 / (1.0 + np.exp(-opacities.astype(np.float64)))
    colv = 1.0 / (1.0 + np.exp(-rgbs.astype(np.float64)[:, 0]))

    # reference sorts by fp32 camera z ascending (stable); we composite
    # back-to-front = exact reverse
    order = np.argsort(z.astype(np.float32), kind="stable")
    rev = order[::-1]
    return (ca[rev], cb[rev], cc[rev], u[rev], v[rev],
            np.log(op)[rev], colv[rev])


def _block_masks(ca, cb, cc, u, v):
    """Exact minimal sigma over each block rectangle: 0 if the center is
    inside, else the min over the four edges (1D quadratic, clamped)."""
    def sigma_at(dx, dy):
        return 0.5 * ca * dx * dx + cb * dx * dy + 0.5 * cc * dy * dy

    masks = np.zeros((NBLK, N), bool)
    for by in range(NBY):
        y0, y1 = by * BR + 0.5, by * BR + BR - 0.5
        for bx in range(NBX):
            x0, x1 = bx * BC + 0.5, bx * BC + BC - 0.5
            smin = np.full(N, np.inf)
            for xe in (x0, x1):
                dxe = xe - u
                dye = np.clip(-cb * dxe / cc, y0 - v, y1 - v)
                smin = np.minimum(smin, sigma_at(dxe, dye))
            for ye in (y0, y1):
                dye = ye - v
                dxe = np.clip(-cb * dye / ca, x0 - u, x1 - u)
                smin = np.minimum(smin, sigma_at(dxe, dye))
            inside = (u >= x0) & (u <= x1) & (v >= y0) & (v <= y1)
            smin[inside] = 0.0
            masks[by * NBX + bx] = smin < CULL_T
    return masks


def _local_basis():
    """[12, 128] block-local monomial basis (exact in bf16)."""
    px = np.arange(BC) + 0.5 - BC / 2.0          # -7.5 .. 7.5
    py = np.arange(BR) + 0.5 - BR / 2.0          # -3.5 .. 3.5
    gx, gy = np.meshgrid(px, py)                 # [BR, BC] row-major
    fx, fy = gx.ravel(), gy.ravel()
    rows = [fx * fx, fx * fy, fy * fy, fx, fy, np.ones_like(fx)]
    return np.repeat(np.stack(rows, 0), 2, axis=0)  # each row duplicated


def _build_schedule(ca, cb, cc, u, v, lop, colv):
    """Cull per block, snake-deal blocks to cores, bin-pack padded slots
    into independent 512-col chunks, build per-core device arrays."""
    masks = _block_masks(ca, cb, cc, u, v)
    widths = masks.sum(1)

    order = np.argsort(widths, kind="stable")[::-1]
    blk_of = np.zeros((NCORES, SLOTS), np.int32)
    for j in range(SLOTS):
        grp = order[j * NCORES:(j + 1) * NCORES]
        if j % 2 == 1:
            grp = grp[::-1]
        blk_of[:, j] = grp
    slot_w = widths[blk_of].max(0)                       # shared schedule
    units = (slot_w + 1 + QU - 1) // QU                  # >=1 sentinel col

    # --- pack: oversize slots get dedicated consecutive chunks (scan
    # carries across them); regular slots first-fit-decreasing into bins
    desc = sorted(range(SLOTS), key=lambda j: -units[j])
    place = np.zeros(SLOTS, np.int64)                    # absolute unit start
    carry = []                                           # per chunk
    abs_u = 0
    regular = []
    for j in desc:
        uj = int(units[j])
        if uj > CAP:
            run = -(-uj // CAP)
            place[j] = abs_u + run * CAP - uj
            carry.extend([False] + [True] * (run - 1))
            abs_u += run * CAP
        else:
            regular.append(j)
    fills, members = [], []                              # per bin
    for j in regular:
        uj = int(units[j])
        for i in range(len(fills)):
            if fills[i] + uj <= CAP:
                place[j] = fills[i]                      # offset; base later
                members[i].append(j)
                fills[i] += uj
                break
        else:
            place[j] = 0
            members.append([j])
            fills.append(uj)
    for i in range(len(fills)):
        for j in members[i]:
            place[j] += abs_u + i * CAP
        carry.append(False)
    abs_u += len(fills) * CAP
    Lpad = abs_u * QU
    S = Lpad // 512
    assert len(carry) == S

    bf16 = ml_dtypes.bfloat16
    cores = []
    c_last = np.zeros((NCORES, SLOTS), np.float64)
    for cid in range(NCORES):
        g12 = np.zeros((12, Lpad), np.float64)
        delta = np.zeros(Lpad, np.float32)
        for j in range(SLOTS):
            blk = int(blk_of[cid, j])
            idx = np.nonzero(masks[blk])[0]
            nb = len(idx)
            if nb == 0:
                continue
            end = (int(place[j]) + int(units[j])) * QU
            s0 = end - nb
            by, bx = divmod(blk, NBX)
            cxb = bx * BC + BC / 2.0
            cyb = by * BR + BR / 2.0
            du = u[idx] - cxb
            dv = v[idx] - cyb
            cai, cbi, cci = ca[idx], cb[idx], cc[idx]
            gs = [-0.5 * cai, -cbi, -0.5 * cci,
                  cai * du + cbi * dv, cbi * du + cci * dv,
                  lop[idx] - (0.5 * cai * du * du + cbi * du * dv
                              + 0.5 * cci * dv * dv)]
            for r, g in enumerate(gs):
                hi = np.asarray(g, dtype=bf16).astype(np.float64)
                lo = g - hi
                g12[2 * r, s0:end] = hi
                g12[2 * r + 1, s0:end] = lo
            cv = colv[idx]
            delta[s0:end] = (cv - np.concatenate([[0.0], cv[:-1]])
                             ).astype(np.float32)
            c_last[cid, j] = cv[-1]
        cores.append({"g12": g12.astype(bf16), "delta": delta})
    sched = {"blk_of": blk_of, "place": place, "units": units,
             "carry": tuple(carry), "Lpad": int(Lpad), "c_last": c_last}
    return sched, cores


# ---------------------------------------------------------------- device

def _build_module(Lpad, carry, reps=1, loop_n=1, stagger=True):
    import contextlib
    import concourse.bass as bass
    import concourse.bacc as bacc
    import concourse.tile as tile
    from concourse import mybir

    f32 = mybir.dt.float32
    bf16 = mybir.dt.bfloat16
    S = Lpad // 512
    U = Lpad // QU

    nc = bacc.Bacc(None)
    ft = nc.dram_tensor("ft", [12, 128], bf16, kind="ExternalInput")
    g12 = nc.dram_tensor("g12", [12, Lpad], bf16, kind="ExternalInput")
    delta = nc.dram_tensor("delta", [Lpad], f32, kind="ExternalInput")
    out = nc.dram_tensor("out", [128 * U], f32, kind="ExternalOutput")

    with tile.TileContext(nc) as tc:
        with (
            tc.tile_pool(name="const", bufs=1) as consts,
            tc.tile_pool(name="work", bufs=6) as work,
            tc.tile_pool(name="psum", bufs=8, space="PSUM") as psum,
        ):
            ft_s = consts.tile([12, 128], bf16)
            nc.sync.dma_start(out=ft_s[:], in_=ft[:, :])
            g_s = consts.tile([12, Lpad], bf16)
            nc.sync.dma_start(out=g_s[:], in_=g12[:, :])
            d_s = consts.tile([128, Lpad], f32)
            step = Lpad // 8
            for q in range(8):
                seg = delta[q * step:(q + 1) * step]
                bc = bass.AP(tensor=seg.tensor, offset=seg.offset,
                             ap=[[0, 128], seg.ap[0]])
                nc.sync.dma_start(out=d_s[:, q * step:(q + 1) * step], in_=bc)
            D = consts.tile([128, Lpad], f32)
            res = consts.tile([128, U], f32)

            loop_cm = (
                tc.For_i(0, loop_n, 1, hint_engines=(
                    mybir.EngineType.PE, mybir.EngineType.Activation,
                    mybir.EngineType.DVE, mybir.EngineType.Pool),
                    staggered_reset=stagger)
                if loop_n > 1 else contextlib.nullcontext()
            )
            with loop_cm:
                for _ in range(reps):
                    for s in range(S):
                        sl = slice(s * 512, (s + 1) * 512)
                        m_ps = psum.tile([128, 512], f32)
                        nc.tensor.matmul(
                            m_ps[:, :], lhsT=ft_s[:, :], rhs=g_s[:, sl],
                            start=True, stop=True,
                        )
                        a_t = work.tile([128, 512], f32)
                        nc.scalar.activation(
                            out=a_t[:], in_=m_ps[:, :],
                            func=mybir.ActivationFunctionType.Exp,
                            scale=1.0, bias=0.0,
                        )
                        am_t = work.tile([128, 512], f32)
                        nc.vector.tensor_scalar_add(am_t[:], a_t[:], -1.0)
                        init = (D[:, s * 512 - 1:s * 512] if carry[s]
                                else 0.0)
                        nc.vector.tensor_tensor_scan(
                            D[:, sl], d_s[:, sl], am_t[:], init,
                            op0=mybir.AluOpType.subtract,
                            op1=mybir.AluOpType.mult,
                        )
                    dfull = D[:]
                    strided = bass.AP(
                        tensor=dfull.tensor, offset=dfull.offset + (QU - 1),
                        ap=[dfull.ap[0], [QU, U]])
                    nc.scalar.copy(out=res[:], in_=strided)
                    nc.sync.dma_start(
                        out=out[:].rearrange("(k c) -> k c", c=U),
                        in_=res[:])
    nc.finalize()
    return nc


# ---------------------------------------------------------------- entry

def _prepare(inputs, reps=1, loop_n=1, variant=None, stagger=True):
    ca, cb, cc, u, v, lop, colv = _preprocess(**inputs)
    sched, cores = _build_schedule(ca, cb, cc, u, v, lop, colv)
    key = (sched["carry"], sched["Lpad"], reps, loop_n, stagger)
    if key not in _cache:
        _cache[key] = _build_module(
            sched["Lpad"], sched["carry"], reps=reps, loop_n=loop_n,
            stagger=stagger)
    nc = _cache[key]
    ftb = _local_basis().astype(ml_dtypes.bfloat16)
    in_maps = [{"ft": ftb, "g12": cores[cid]["g12"],
                "delta": cores[cid]["delta"]} for cid in range(NCORES)]
    return nc, in_maps, sched


def _assemble(results, sched):
    img = np.zeros((H, W), np.float32)
    blk_of = sched["blk_of"]
    place, units = sched["place"], sched["units"]
    U = sched["Lpad"] // QU
    for cid in range(NCORES):
        res = results[cid]["out"].reshape(128, U)
        for j in range(SLOTS):
            by, bx = divmod(int(blk_of[cid, j]), NBX)
            uend = int(place[j]) + int(units[j]) - 1
            col = res[:, uend] + np.float32(sched["c_last"][cid, j])
            img[by * BR:(by + 1) * BR, bx * BC:(bx + 1) * BC] = (
                col.reshape(BR, BC))
    return img.reshape(1, 1, H, W)


def kernel(**inputs):
    from concourse.bass_utils import run_bass_kernel_spmd

    inputs = {k: np.asarray(v) for k, v in inputs.items()}
    nc, in_maps, sched = _prepare(inputs)
    res = run_bass_kernel_spmd(nc, in_maps, core_ids=list(range(NCORES)))
    return _assemble(res.results, sched)


# revision 13
# speedup vs baseline: 4.0260x; 1.2746x over previous
"""Trainium2 Bass kernel for 2D Gaussian Splatting (N=1024 gaussians, 256x256).

Math: sigma[p,i] is a quadratic polynomial in pixel coords, so with a
block-centered pixel basis F[12,128] (6 monomials, each duplicated for a
hi/lo bf16 coefficient split) m1 = log(op) - sigma is ONE bf16 matmul
F.T @ G per 512-column chunk (the basis is block-independent in local
coords, so a single lhsT serves the whole stream). alpha = exp(m1) on
the scalar engine; a = alpha - 1 on gpsimd/DVE; compositing uses the
shifted state D = C - c (c = per-gaussian color) which obeys
    D_k = (delta_k - D_{k-1}) * a_k,   delta_k = c_k - c_{k-1},
one DVE tensor_tensor_scan (op0=subtract, op1=mult) per chunk with no
separate beta/bt tensors. delta is a host-precomputed constant. The host
adds back c at each slot's final column.

Culling: image split into 512 blocks of 8x16 pixels; a gaussian is kept
for a block iff its exact minimal sigma over the block is < 8 (dropped
alpha contributions < 2e-4 rel). Blocks are snake-dealt by surviving
count onto the 8 cores (SPMD: identical slot schedule, data-dependent
content only). Slots are front-padded with sentinel columns (all-zero
coefficients -> m1=0 -> a=0 resets the scan) to a multiple of 16
columns and bin-packed into independent 512-column chunks; only slots
wider than 512 chain scan state across their own consecutive chunks.
Slot composites are extracted with one strided copy (every 16th column)
+ one DMA; the host picks each slot's end column.

Sharding: 8 NeuronCores; gaussian params replicated, blocks balanced;
host reassembles the image from the per-core strided outputs.
"""

import numpy as np
import ml_dtypes

H = 256
W = 256
N = 1024
NCORES = 8
BR, BC = 8, 16                 # block = 8 rows x 16 cols = 128 pixels
NBY, NBX = H // BR, W // BC
NBLK = NBY * NBX               # 512
SLOTS = NBLK // NCORES         # 64 slots per core
CULL_T = 5.0
QU = 8                         # unit quantum (columns); slot ends at u*QU-1
CAP = 512 // QU                # units per chunk
EPS2D = 0.3

_cache = {}


# ---------------------------------------------------------------- host math

def _preprocess(means, quats, scales, rgbs, opacities, viewmat, K):
    """Float64 per-gaussian preprocessing. Returns, in back-to-front order:
    conic (ca, cb, cc), pixel means (u, v), log-opacity lop, colors colv."""
    md = means.astype(np.float64)
    Rv = viewmat[:3, :3].astype(np.float64)
    t = viewmat[:3, 3].astype(np.float64)
    p_cam = md @ Rv.T + t
    x, y, z = p_cam[:, 0], p_cam[:, 1], p_cam[:, 2]
    fx, fy = float(K[0, 0]), float(K[1, 1])
    cx, cy = float(K[0, 2]), float(K[1, 2])
    inv_z = 1.0 / z
    u = fx * x * inv_z + cx
    v = fy * y * inv_z + cy

    th = quats.astype(np.float64)
    ct, st = np.cos(th), np.sin(th)
    zr = np.zeros_like(ct)
    R3 = np.stack([np.stack([ct, -st, zr], -1),
                   np.stack([st, ct, zr], -1),
                   np.stack([zr, zr, np.ones_like(ct)], -1)], -2)
    M = R3 * scales.astype(np.float64)[:, None, :]
    cov3 = M @ np.swapaxes(M, -1, -2)
    cov_cam = np.einsum('ij,njk,lk->nil', Rv, cov3, Rv)
    j0 = np.stack([fx * inv_z, zr, -fx * x * inv_z * inv_z], -1)
    j1 = np.stack([zr, fy * inv_z, -fy * y * inv_z * inv_z], -1)
    J = np.stack([j0, j1], -2)
    cov2 = np.einsum('nij,njk,nlk->nil', J, cov_cam, J)
    a = cov2[:, 0, 0] + EPS2D
    b = cov2[:, 0, 1]
    c = cov2[:, 1, 1] + EPS2D
    det = a * c - b * b
    ca, cb, cc = c / det, -b / det, a / det

    op = 1.0 / (1.0 + np.exp(-opacities.astype(np.float64)))
    colv = 1.0 / (1.0 + np.exp(-rgbs.astype(np.float64)[:, 0]))

    # reference sorts by fp32 camera z ascending (stable); we composite
    # back-to-front = exact reverse
    order = np.argsort(z.astype(np.float32), kind="stable")
    rev = order[::-1]
    return (ca[rev], cb[rev], cc[rev], u[rev], v[rev],
            np.log(op)[rev], colv[rev])


def _block_masks(ca, cb, cc, u, v):
    """Exact minimal sigma over each block rectangle: 0 if the center is
    inside, else the min over the four edges (1D quadratic, clamped)."""
    def sigma_at(dx, dy):
        return 0.5 * ca * dx * dx + cb * dx * dy + 0.5 * cc * dy * dy

    masks = np.zeros((NBLK, N), bool)
    for by in range(NBY):
        y0, y1 = by * BR + 0.5, by * BR + BR - 0.5
        for bx in range(NBX):
            x0, x1 = bx * BC + 0.5, bx * BC + BC - 0.5
            smin = np.full(N, np.inf)
            for xe in (x0, x1):
                dxe = xe - u
                dye = np.clip(-cb * dxe / cc, y0 - v, y1 - v)
                smin = np.minimum(smin, sigma_at(dxe, dye))
            for ye in (y0, y1):
                dye = ye - v
                dxe = np.clip(-cb * dye / ca, x0 - u, x1 - u)
                smin = np.minimum(smin, sigma_at(dxe, dye))
            inside = (u >= x0) & (u <= x1) & (v >= y0) & (v <= y1)
            smin[inside] = 0.0
            masks[by * NBX + bx] = smin < CULL_T
    return masks


def _local_basis():
    """[12, 128] block-local monomial basis (exact in bf16)."""
    px = np.arange(BC) + 0.5 - BC / 2.0          # -7.5 .. 7.5
    py = np.arange(BR) + 0.5 - BR / 2.0          # -3.5 .. 3.5
    gx, gy = np.meshgrid(px, py)                 # [BR, BC] row-major
    fx, fy = gx.ravel(), gy.ravel()
    rows = [fx * fx, fx * fy, fy * fy, fx, fy, np.ones_like(fx)]
    return np.repeat(np.stack(rows, 0), 2, axis=0)  # each row duplicated


def _build_schedule(ca, cb, cc, u, v, lop, colv):
    """Cull per block, snake-deal blocks to cores, bin-pack padded slots
    into independent 512-col chunks, build per-core device arrays."""
    masks = _block_masks(ca, cb, cc, u, v)
    widths = masks.sum(1)

    order = np.argsort(widths, kind="stable")[::-1]
    blk_of = np.zeros((NCORES, SLOTS), np.int32)
    for j in range(SLOTS):
        grp = order[j * NCORES:(j + 1) * NCORES]
        if j % 2 == 1:
            grp = grp[::-1]
        blk_of[:, j] = grp
    slot_w = widths[blk_of].max(0)                       # shared schedule
    units = (slot_w + 1 + QU - 1) // QU                  # >=1 sentinel col

    # --- pack: oversize slots get dedicated consecutive chunks (scan
    # carries across them); regular slots first-fit-decreasing into bins
    desc = sorted(range(SLOTS), key=lambda j: -units[j])
    place = np.zeros(SLOTS, np.int64)                    # absolute unit start
    carry = []                                           # per chunk
    abs_u = 0
    regular = []
    for j in desc:
        uj = int(units[j])
        if uj > CAP:
            run = -(-uj // CAP)
            place[j] = abs_u + run * CAP - uj
            carry.extend([False] + [True] * (run - 1))
            abs_u += run * CAP
        else:
            regular.append(j)
    fills, members = [], []                              # per bin
    for j in regular:
        uj = int(units[j])
        for i in range(len(fills)):
            if fills[i] + uj <= CAP:
                place[j] = fills[i]                      # offset; base later
                members[i].append(j)
                fills[i] += uj
                break
        else:
            place[j] = 0
            members.append([j])
            fills.append(uj)
    for i in range(len(fills)):
        for j in members[i]:
            place[j] += abs_u + i * CAP
        carry.append(False)
    abs_u += len(fills) * CAP
    Lpad = abs_u * QU
    S = Lpad // 512
    assert len(carry) == S

    bf16 = ml_dtypes.bfloat16
    cores = []
    c_last = np.zeros((NCORES, SLOTS), np.float64)
    for cid in range(NCORES):
        g12 = np.zeros((12, Lpad), np.float64)
        delta = np.zeros(Lpad, np.float32)
        for j in range(SLOTS):
            blk = int(blk_of[cid, j])
            idx = np.nonzero(masks[blk])[0]
            nb = len(idx)
            if nb == 0:
                continue
            end = (int(place[j]) + int(units[j])) * QU
            s0 = end - nb
            by, bx = divmod(blk, NBX)
            cxb = bx * BC + BC / 2.0
            cyb = by * BR + BR / 2.0
            du = u[idx] - cxb
            dv = v[idx] - cyb
            cai, cbi, cci = ca[idx], cb[idx], cc[idx]
            gs = [-0.5 * cai, -cbi, -0.5 * cci,
                  cai * du + cbi * dv, cbi * du + cci * dv,
                  lop[idx] - (0.5 * cai * du * du + cbi * du * dv
                              + 0.5 * cci * dv * dv)]
            for r, g in enumerate(gs):
                hi = np.asarray(g, dtype=bf16).astype(np.float64)
                lo = g - hi
                g12[2 * r, s0:end] = hi
                g12[2 * r + 1, s0:end] = lo
            cv = colv[idx]
            delta[s0:end] = (cv - np.concatenate([[0.0], cv[:-1]])
                             ).astype(np.float32)
            c_last[cid, j] = cv[-1]
        cores.append({"g12": g12.astype(bf16), "delta": delta})
    sched = {"blk_of": blk_of, "place": place, "units": units,
             "carry": tuple(carry), "Lpad": int(Lpad), "c_last": c_last}
    return sched, cores


# ---------------------------------------------------------------- device

def _build_module(Lpad, carry, reps=1, loop_n=1, stagger=True):
    import contextlib
    import concourse.bass as bass
    import concourse.bacc as bacc
    import concourse.tile as tile
    from concourse import mybir

    f32 = mybir.dt.float32
    bf16 = mybir.dt.bfloat16
    S = Lpad // 512
    U = Lpad // QU

    nc = bacc.Bacc(None)
    ft = nc.dram_tensor("ft", [12, 128], bf16, kind="ExternalInput")
    g12 = nc.dram_tensor("g12", [12, Lpad], bf16, kind="ExternalInput")
    delta = nc.dram_tensor("delta", [Lpad], f32, kind="ExternalInput")
    out = nc.dram_tensor("out", [128 * U], f32, kind="ExternalOutput")

    with tile.TileContext(nc) as tc:
        with (
            tc.tile_pool(name="const", bufs=1) as consts,
            tc.tile_pool(name="work", bufs=6) as work,
            tc.tile_pool(name="psum", bufs=8, space="PSUM") as psum,
        ):
            ft_s = consts.tile([12, 128], bf16)
            nc.sync.dma_start(out=ft_s[:], in_=ft[:, :])
            g_s = consts.tile([12, Lpad], bf16)
            nc.sync.dma_start(out=g_s[:], in_=g12[:, :])
            d_s = consts.tile([128, Lpad], f32)
            step = Lpad // 8
            for q in range(8):
                seg = delta[q * step:(q + 1) * step]
                bc = bass.AP(tensor=seg.tensor, offset=seg.offset,
                             ap=[[0, 128], seg.ap[0]])
                nc.sync.dma_start(out=d_s[:, q * step:(q + 1) * step], in_=bc)
            D = consts.tile([128, Lpad], f32)
            res = consts.tile([128, U], f32)

            loop_cm = (
                tc.For_i(0, loop_n, 1, hint_engines=(
                    mybir.EngineType.PE, mybir.EngineType.Activation,
                    mybir.EngineType.DVE, mybir.EngineType.Pool),
                    staggered_reset=stagger)
                if loop_n > 1 else contextlib.nullcontext()
            )
            with loop_cm:
                for _ in range(reps):
                    for s in range(S):
                        sl = slice(s * 512, (s + 1) * 512)
                        m_ps = psum.tile([128, 512], f32)
                        nc.tensor.matmul(
                            m_ps[:, :], lhsT=ft_s[:, :], rhs=g_s[:, sl],
                            start=True, stop=True,
                        )
                        a_t = work.tile([128, 512], f32)
                        nc.scalar.activation(
                            out=a_t[:], in_=m_ps[:, :],
                            func=mybir.ActivationFunctionType.Exp,
                            scale=1.0, bias=0.0,
                        )
                        am_t = work.tile([128, 512], f32)
                        nc.vector.tensor_scalar_add(am_t[:], a_t[:], -1.0)
                        init = (D[:, s * 512 - 1:s * 512] if carry[s]
                                else 0.0)
                        nc.vector.tensor_tensor_scan(
                            D[:, sl], d_s[:, sl], am_t[:], init,
                            op0=mybir.AluOpType.subtract,
                            op1=mybir.AluOpType.mult,
                        )
                        if not (s + 1 < S and carry[s + 1]):
                            # extract this chunk's slot-end columns now, so
                            # the next iteration's scan of this chunk only
                            # waits on this small copy, not a full-D read
                            uc = 512 // QU
                            dsl = D[:, sl]
                            strided = bass.AP(
                                tensor=dsl.tensor,
                                offset=dsl.offset + (QU - 1),
                                ap=[dsl.ap[0], [QU, uc]])
                            nc.scalar.copy(
                                out=res[:, s * uc:(s + 1) * uc], in_=strided)
                    nc.sync.dma_start(
                        out=out[:].rearrange("(k c) -> k c", c=U),
                        in_=res[:])
    nc.finalize()
    return nc


# ---------------------------------------------------------------- entry

def _prepare(inputs, reps=1, loop_n=1, variant=None, stagger=True):
    ca, cb, cc, u, v, lop, colv = _preprocess(**inputs)
    sched, cores = _build_schedule(ca, cb, cc, u, v, lop, colv)
    key = (sched["carry"], sched["Lpad"], reps, loop_n, stagger)
    if key not in _cache:
        _cache[key] = _build_module(
            sched["Lpad"], sched["carry"], reps=reps, loop_n=loop_n,
            stagger=stagger)
    nc = _cache[key]
    ftb = _local_basis().astype(ml_dtypes.bfloat16)
    in_maps = [{"ft": ftb, "g12": cores[cid]["g12"],
                "delta": cores[cid]["delta"]} for cid in range(NCORES)]
    return nc, in_maps, sched


def _assemble(results, sched):
    img = np.zeros((H, W), np.float32)
    blk_of = sched["blk_of"]
    place, units = sched["place"], sched["units"]
    U = sched["Lpad"] // QU
    for cid in range(NCORES):
        res = results[cid]["out"].reshape(128, U)
        for j in range(SLOTS):
            by, bx = divmod(int(blk_of[cid, j]), NBX)
            uend = int(place[j]) + int(units[j]) - 1
            col = res[:, uend] + np.float32(sched["c_last"][cid, j])
            img[by * BR:(by + 1) * BR, bx * BC:(bx + 1) * BC] = (
                col.reshape(BR, BC))
    return img.reshape(1, 1, H, W)


def kernel(**inputs):
    from concourse.bass_utils import run_bass_kernel_spmd

    inputs = {k: np.asarray(v) for k, v in inputs.items()}
    nc, in_maps, sched = _prepare(inputs)
    res = run_bass_kernel_spmd(nc, in_maps, core_ids=list(range(NCORES)))
    return _assemble(res.results, sched)


# revision 17
# speedup vs baseline: 10.2369x; 2.5427x over previous
"""Trainium2 Bass kernel for 2D Gaussian Splatting (N=1024 gaussians, 256x256).

Math: sigma[p,i] is a quadratic polynomial in pixel coords, so with a
block-centered pixel basis F[12,128] (6 monomials, each duplicated for a
hi/lo bf16 coefficient split) m1 = log(op) - sigma is ONE bf16 matmul
F.T @ G per 512-column chunk (the basis is block-independent in local
coords, so a single lhsT serves the whole stream). alpha = exp(m1) on
the scalar engine; a = alpha - 1 on gpsimd/DVE; compositing uses the
shifted state D = C - c (c = per-gaussian color) which obeys
    D_k = (delta_k - D_{k-1}) * a_k,   delta_k = c_k - c_{k-1},
one DVE tensor_tensor_scan (op0=subtract, op1=mult) per chunk with no
separate beta/bt tensors. delta is a host-precomputed constant. The host
adds back c at each slot's final column.

Culling: image split into 512 blocks of 8x16 pixels; a gaussian is kept
for a block iff its exact minimal sigma over the block is < CULL_T
(rel err ~3e-3 at CULL_T=5 vs the 2e-2 gate). Blocks are snake-dealt by
surviving count onto the 8 cores (SPMD: identical slot schedule,
data-dependent content only). Slots are front-padded with sentinel
columns (all-zero coefficients -> m1=0 -> a=0 resets the scan) to a
multiple of QU columns and bin-packed into independent 512-column
chunks; only slots wider than 512 chain scan state across their own
consecutive chunks. Each chunk's slot-end columns are extracted with a
per-chunk strided copy right after its scan (so the next loop
iteration's scan never waits on a whole-D read) + one DMA; the host
picks each slot's end column. The For_i timing loop uses
staggered_reset to overlap iterations instead of a drain barrier.
gpsimd (Pool) is deliberately unused: a [128,512] tensor_scalar
measured ~8.5us there vs ~0.19us on DVE.

Sharding: 8 NeuronCores; gaussian params replicated, blocks balanced;
host reassembles the image from the per-core strided outputs.
"""

import numpy as np
import ml_dtypes

H = 256
W = 256
N = 1024
NCORES = 8
BR, BC = 8, 16                 # block = 8 rows x 16 cols = 128 pixels
NBY, NBX = H // BR, W // BC
NBLK = NBY * NBX               # 512
SLOTS = NBLK // NCORES         # 64 slots per core
CULL_T = 5.0
QU = 8                         # unit quantum (columns); slot ends at u*QU-1
CAP = 512 // QU                # units per chunk
EPS2D = 0.3

_cache = {}


# ---------------------------------------------------------------- host math

def _preprocess(means, quats, scales, rgbs, opacities, viewmat, K):
    """Float64 per-gaussian preprocessing. Returns, in back-to-front order:
    conic (ca, cb, cc), pixel means (u, v), log-opacity lop, colors colv."""
    md = means.astype(np.float64)
    Rv = viewmat[:3, :3].astype(np.float64)
    t = viewmat[:3, 3].astype(np.float64)
    p_cam = md @ Rv.T + t
    x, y, z = p_cam[:, 0], p_cam[:, 1], p_cam[:, 2]
    fx, fy = float(K[0, 0]), float(K[1, 1])
    cx, cy = float(K[0, 2]), float(K[1, 2])
    inv_z = 1.0 / z
    u = fx * x * inv_z + cx
    v = fy * y * inv_z + cy

    th = quats.astype(np.float64)
    ct, st = np.cos(th), np.sin(th)
    zr = np.zeros_like(ct)
    R3 = np.stack([np.stack([ct, -st, zr], -1),
                   np.stack([st, ct, zr], -1),
                   np.stack([zr, zr, np.ones_like(ct)], -1)], -2)
    M = R3 * scales.astype(np.float64)[:, None, :]
    cov3 = M @ np.swapaxes(M, -1, -2)
    cov_cam = np.einsum('ij,njk,lk->nil', Rv, cov3, Rv)
    j0 = np.stack([fx * inv_z, zr, -fx * x * inv_z * inv_z], -1)
    j1 = np.stack([zr, fy * inv_z, -fy * y * inv_z * inv_z], -1)
    J = np.stack([j0, j1], -2)
    cov2 = np.einsum('nij,njk,nlk->nil', J, cov_cam, J)
    a = cov2[:, 0, 0] + EPS2D
    b = cov2[:, 0, 1]
    c = cov2[:, 1, 1] + EPS2D
    det = a * c - b * b
    ca, cb, cc = c / det, -b / det, a / det

    op = 1.0 / (1.0 + np.exp(-opacities.astype(np.float64)))
    colv = 1.0 / (1.0 + np.exp(-rgbs.astype(np.float64)[:, 0]))

    # reference sorts by fp32 camera z ascending (stable); we composite
    # back-to-front = exact reverse
    order = np.argsort(z.astype(np.float32), kind="stable")
    rev = order[::-1]
    return (ca[rev], cb[rev], cc[rev], u[rev], v[rev],
            np.log(op)[rev], colv[rev])


def _block_masks(ca, cb, cc, u, v):
    """Exact minimal sigma over each block rectangle: 0 if the center is
    inside, else the min over the four edges (1D quadratic, clamped)."""
    def sigma_at(dx, dy):
        return 0.5 * ca * dx * dx + cb * dx * dy + 0.5 * cc * dy * dy

    masks = np.zeros((NBLK, N), bool)
    for by in range(NBY):
        y0, y1 = by * BR + 0.5, by * BR + BR - 0.5
        for bx in range(NBX):
            x0, x1 = bx * BC + 0.5, bx * BC + BC - 0.5
            smin = np.full(N, np.inf)
            for xe in (x0, x1):
                dxe = xe - u
                dye = np.clip(-cb * dxe / cc, y0 - v, y1 - v)
                smin = np.minimum(smin, sigma_at(dxe, dye))
            for ye in (y0, y1):
                dye = ye - v
                dxe = np.clip(-cb * dye / ca, x0 - u, x1 - u)
                smin = np.minimum(smin, sigma_at(dxe, dye))
            inside = (u >= x0) & (u <= x1) & (v >= y0) & (v <= y1)
            smin[inside] = 0.0
            masks[by * NBX + bx] = smin < CULL_T
    return masks


def _local_basis():
    """[12, 128] block-local monomial basis (exact in bf16)."""
    px = np.arange(BC) + 0.5 - BC / 2.0          # -7.5 .. 7.5
    py = np.arange(BR) + 0.5 - BR / 2.0          # -3.5 .. 3.5
    gx, gy = np.meshgrid(px, py)                 # [BR, BC] row-major
    fx, fy = gx.ravel(), gy.ravel()
    rows = [fx * fx, fx * fy, fy * fy, fx, fy, np.ones_like(fx)]
    return np.repeat(np.stack(rows, 0), 2, axis=0)  # each row duplicated


def _build_schedule(ca, cb, cc, u, v, lop, colv):
    """Cull per block, snake-deal blocks to cores, bin-pack padded slots
    into independent 512-col chunks, build per-core device arrays."""
    masks = _block_masks(ca, cb, cc, u, v)
    widths = masks.sum(1)

    order = np.argsort(widths, kind="stable")[::-1]
    blk_of = np.zeros((NCORES, SLOTS), np.int32)
    for j in range(SLOTS):
        grp = order[j * NCORES:(j + 1) * NCORES]
        if j % 2 == 1:
            grp = grp[::-1]
        blk_of[:, j] = grp
    slot_w = widths[blk_of].max(0)                       # shared schedule
    units = (slot_w + 1 + QU - 1) // QU                  # >=1 sentinel col

    # --- pack: oversize slots get dedicated consecutive chunks (scan
    # carries across them); regular slots first-fit-decreasing into bins
    desc = sorted(range(SLOTS), key=lambda j: -units[j])
    place = np.zeros(SLOTS, np.int64)                    # absolute unit start
    carry = []                                           # per chunk
    abs_u = 0
    regular = []
    for j in desc:
        uj = int(units[j])
        if uj > CAP:
            run = -(-uj // CAP)
            place[j] = abs_u + run * CAP - uj
            carry.extend([False] + [True] * (run - 1))
            abs_u += run * CAP
        else:
            regular.append(j)
    fills, members = [], []                              # per bin
    for j in regular:
        uj = int(units[j])
        for i in range(len(fills)):
            if fills[i] + uj <= CAP:
                place[j] = fills[i]                      # offset; base later
                members[i].append(j)
                fills[i] += uj
                break
        else:
            place[j] = 0
            members.append([j])
            fills.append(uj)
    for i in range(len(fills)):
        for j in members[i]:
            place[j] += abs_u + i * CAP
        carry.append(False)
    abs_u += len(fills) * CAP
    Lpad = abs_u * QU
    S = Lpad // 512
    assert len(carry) == S

    bf16 = ml_dtypes.bfloat16
    cores = []
    c_last = np.zeros((NCORES, SLOTS), np.float64)
    for cid in range(NCORES):
        g12 = np.zeros((12, Lpad), np.float64)
        delta = np.zeros(Lpad, np.float32)
        for j in range(SLOTS):
            blk = int(blk_of[cid, j])
            idx = np.nonzero(masks[blk])[0]
            nb = len(idx)
            if nb == 0:
                continue
            end = (int(place[j]) + int(units[j])) * QU
            s0 = end - nb
            by, bx = divmod(blk, NBX)
            cxb = bx * BC + BC / 2.0
            cyb = by * BR + BR / 2.0
            du = u[idx] - cxb
            dv = v[idx] - cyb
            cai, cbi, cci = ca[idx], cb[idx], cc[idx]
            gs = [-0.5 * cai, -cbi, -0.5 * cci,
                  cai * du + cbi * dv, cbi * du + cci * dv,
                  lop[idx] - (0.5 * cai * du * du + cbi * du * dv
                              + 0.5 * cci * dv * dv)]
            for r, g in enumerate(gs):
                hi = np.asarray(g, dtype=bf16).astype(np.float64)
                lo = g - hi
                g12[2 * r, s0:end] = hi
                g12[2 * r + 1, s0:end] = lo
            cv = colv[idx]
            delta[s0:end] = (cv - np.concatenate([[0.0], cv[:-1]])
                             ).astype(np.float32)
            c_last[cid, j] = cv[-1]
        cores.append({"g12": g12.astype(bf16), "delta": delta})
    sched = {"blk_of": blk_of, "place": place, "units": units,
             "carry": tuple(carry), "Lpad": int(Lpad), "c_last": c_last}
    return sched, cores


# ---------------------------------------------------------------- device

def _build_module(Lpad, carry, reps=1, loop_n=1, stagger=True):
    import contextlib
    import concourse.bass as bass
    import concourse.bacc as bacc
    import concourse.tile as tile
    from concourse import mybir

    f32 = mybir.dt.float32
    bf16 = mybir.dt.bfloat16
    S = Lpad // 512
    U = Lpad // QU

    nc = bacc.Bacc(None)
    ft = nc.dram_tensor("ft", [12, 128], bf16, kind="ExternalInput")
    g12 = nc.dram_tensor("g12", [12, Lpad], bf16, kind="ExternalInput")
    delta = nc.dram_tensor("delta", [Lpad], f32, kind="ExternalInput")
    out = nc.dram_tensor("out", [128 * U], f32, kind="ExternalOutput")

    with tile.TileContext(nc) as tc:
        with (
            tc.tile_pool(name="const", bufs=1) as consts,
            tc.tile_pool(name="work", bufs=12) as work,
            tc.tile_pool(name="psum", bufs=8, space="PSUM") as psum,
        ):
            ft_s = consts.tile([12, 128], bf16)
            nc.sync.dma_start(out=ft_s[:], in_=ft[:, :])
            g_s = consts.tile([12, Lpad], bf16)
            nc.sync.dma_start(out=g_s[:], in_=g12[:, :])
            d_s = consts.tile([128, Lpad], f32)
            step = Lpad // 8
            for q in range(8):
                seg = delta[q * step:(q + 1) * step]
                bc = bass.AP(tensor=seg.tensor, offset=seg.offset,
                             ap=[[0, 128], seg.ap[0]])
                nc.sync.dma_start(out=d_s[:, q * step:(q + 1) * step], in_=bc)
            D = consts.tile([128, Lpad], f32)
            res = consts.tile([128, U], f32)

            loop_cm = (
                tc.For_i(0, loop_n, 1, hint_engines=(
                    mybir.EngineType.PE, mybir.EngineType.Activation,
                    mybir.EngineType.DVE, mybir.EngineType.Pool),
                    staggered_reset=stagger)
                if loop_n > 1 else contextlib.nullcontext()
            )
            with loop_cm:
                for _ in range(reps):
                    for s in range(S):
                        sl = slice(s * 512, (s + 1) * 512)
                        m_ps = psum.tile([128, 512], f32)
                        nc.tensor.matmul(
                            m_ps[:, :], lhsT=ft_s[:, :], rhs=g_s[:, sl],
                            start=True, stop=True,
                        )
                        a_t = work.tile([128, 512], f32)
                        nc.scalar.activation(
                            out=a_t[:], in_=m_ps[:, :],
                            func=mybir.ActivationFunctionType.Exp,
                            scale=1.0, bias=0.0,
                        )
                        am_t = work.tile([128, 512], f32)
                        # a-1 on Act (Copy applies in*scale+bias); keeps DVE
                        # a pure scan stream
                        nc.scalar.activation(
                            out=am_t[:], in_=a_t[:],
                            func=mybir.ActivationFunctionType.Copy,
                            scale=1.0, bias=-1.0,
                        )
                        init = (D[:, s * 512 - 1:s * 512] if carry[s]
                                else 0.0)
                        nc.vector.tensor_tensor_scan(
                            D[:, sl], d_s[:, sl], am_t[:], init,
                            op0=mybir.AluOpType.subtract,
                            op1=mybir.AluOpType.mult,
                        )
                    # per-chunk strided extracts, emitted after the chunk
                    # loop: Act executes in order, so a copy waiting on a
                    # scan must not sit between exp/add pairs
                    for s in range(S):
                        if s + 1 < S and carry[s + 1]:
                            continue
                        uc = 512 // QU
                        dsl = D[:, s * 512:(s + 1) * 512]
                        strided = bass.AP(
                            tensor=dsl.tensor,
                            offset=dsl.offset + (QU - 1),
                            ap=[dsl.ap[0], [QU, uc]])
                        nc.scalar.copy(
                            out=res[:, s * uc:(s + 1) * uc], in_=strided)
                    nc.sync.dma_start(
                        out=out[:].rearrange("(k c) -> k c", c=U),
                        in_=res[:])
    nc.finalize()
    return nc


# ---------------------------------------------------------------- entry

def _prepare(inputs, reps=1, loop_n=1, variant=None, stagger=True):
    ca, cb, cc, u, v, lop, colv = _preprocess(**inputs)
    sched, cores = _build_schedule(ca, cb, cc, u, v, lop, colv)
    key = (sched["carry"], sched["Lpad"], reps, loop_n, stagger)
    if key not in _cache:
        _cache[key] = _build_module(
            sched["Lpad"], sched["carry"], reps=reps, loop_n=loop_n,
            stagger=stagger)
    nc = _cache[key]
    ftb = _local_basis().astype(ml_dtypes.bfloat16)
    in_maps = [{"ft": ftb, "g12": cores[cid]["g12"],
                "delta": cores[cid]["delta"]} for cid in range(NCORES)]
    return nc, in_maps, sched


def _assemble(results, sched):
    img = np.zeros((H, W), np.float32)
    blk_of = sched["blk_of"]
    place, units = sched["place"], sched["units"]
    U = sched["Lpad"] // QU
    for cid in range(NCORES):
        res = results[cid]["out"].reshape(128, U)
        for j in range(SLOTS):
            by, bx = divmod(int(blk_of[cid, j]), NBX)
            uend = int(place[j]) + int(units[j]) - 1
            col = res[:, uend] + np.float32(sched["c_last"][cid, j])
            img[by * BR:(by + 1) * BR, bx * BC:(bx + 1) * BC] = (
                col.reshape(BR, BC))
    return img.reshape(1, 1, H, W)


def kernel(**inputs):
    from concourse.bass_utils import run_bass_kernel_spmd

    inputs = {k: np.asarray(v) for k, v in inputs.items()}
    nc, in_maps, sched = _prepare(inputs)
    res = run_bass_kernel_spmd(nc, in_maps, core_ids=list(range(NCORES)))
    return _assemble(res.results, sched)
